# revision 21
# baseline (speedup 1.0000x reference)
"""Deformable 3x3 conv block (offset conv -> modulated bilinear sampling ->
weighted conv -> frozen BN + ReLU) on 8 Trainium2 NeuronCores.

Sharding: 8 cores = (image n in 0..3) x (horizontal half in 0..1); each core
computes 48 output rows of one image. No cross-core communication.

Per-core device pipeline:
  A) offset conv as 9 shifted fp16 matmuls accumulated in PSUM -> om [27, 4608]
  B) PE-transpose om to position-major omT [128pos, 36tile*27ch] fp32
  C) coordinate/weight math on DVE/ACT (magic-number floor, clamps, validity,
     sigmoid mask) -> 4 corner weights [128, 324] fp32 + int32 pixel indices
  D) indirect-DMA gather of 2-pixel fp16 rows (bilinear corner pairs) from a
     position-major [9218, 256] copy of x in DRAM
  E) blend: tensor_scalar mul (per-position scalar) + scalar_tensor_tensor
     fused mul-add; corner-pair sum via PE transpose accumulation in PSUM
  F) main matmul over (c,k)=2304 contraction in fp16 -> PSUM fp32
  G) frozen-BN + ReLU via ACT activation(scale, bias), store fp32
"""
import sys

for p in ("/opt/trn_rl_repo",):
    if p not in sys.path:
        sys.path.insert(0, p)

import numpy as np
from contextlib import ExitStack

import concourse.bass as bass
import concourse.bacc as bacc
import concourse.mybir as mybir
from concourse.tile import TileContext
from concourse.masks import make_identity
from concourse.bass_utils import run_bass_kernel_spmd

N, CIN, COUT, H, W, K = 4, 256, 256, 96, 96, 9
EPS = 1e-5
HW = H * W           # 9216
NROWS = 48           # rows per core
P = NROWS * W        # 4608 positions per core
NT = P // 128        # 36 position tiles
MAGIC = 12582912.0   # 1.5 * 2^23 (fp32 round-to-int trick)
NSUP = 18            # gather super-chunks (2 position tiles each)

fp16 = mybir.dt.float16
fp32 = mybir.dt.float32
i32 = mybir.dt.int32
i16 = mybir.dt.int16
OP = mybir.AluOpType
AF = mybir.ActivationFunctionType

_PROGRAM_CACHE = {}


def _build_program():
    nc = bacc.Bacc()

    xt = nc.dram_tensor("xt", [HW + 2, CIN], fp16, kind="ExternalInput")
    xcm = nc.dram_tensor("xcm", [2, 128, (NROWS + 2) * (W + 2)], fp16,
                         kind="ExternalInput")
    woff = nc.dram_tensor("woff", [128, 9 * 2 * 27], fp16, kind="ExternalInput")
    w2 = nc.dram_tensor("w2", [128, 2 * 9 * 2 * 128], fp16, kind="ExternalInput")
    base = nc.dram_tensor("base", [128, NT * 27], fp32, kind="ExternalInput")
    sel = nc.dram_tensor("sel", [16, 128], fp32, kind="ExternalInput")
    nscale = nc.dram_tensor("nscale", [128, 2], fp32, kind="ExternalInput")
    nbias = nc.dram_tensor("nbias", [128, 2], fp32, kind="ExternalInput")
    out_d = nc.dram_tensor("out", [COUT, P], fp32, kind="ExternalOutput")

    SLAB = 4           # conv rows per om matmul chunk -> 384 positions
    NSLAB = NROWS // SLAB

    with TileContext(nc) as tc, ExitStack() as ctx:
        const = ctx.enter_context(tc.tile_pool(name="const", bufs=1))
        coords = ctx.enter_context(tc.tile_pool(name="coords", bufs=1))
        gpool = ctx.enter_context(tc.tile_pool(name="gpool", bufs=2))
        bl = ctx.enter_context(tc.tile_pool(name="bl", bufs=3))
        vpool = ctx.enter_context(tc.tile_pool(name="vpool", bufs=2))
        opool = ctx.enter_context(tc.tile_pool(name="opool", bufs=2))

        # ---- constants / inputs to SBUF ----
        xcm_sb = [const.tile([128, NROWS + 2, W + 2], fp16, tag=f"xcm{ch}",
                             name=f"xcm_sb{ch}") for ch in range(2)]
        for ch in range(2):
            nc.sync.dma_start(xcm_sb[ch][:], xcm[ch])
        woff_sb = const.tile([128, 9 * 2 * 27], fp16)
        nc.sync.dma_start(woff_sb[:], woff[:])
        w2_sb = const.tile([128, 2 * 9 * 2 * 128], fp16)
        nc.sync.dma_start(w2_sb[:], w2[:])
        base_sb = const.tile([128, NT * 27], fp32)
        nc.sync.dma_start(base_sb[:], base[:])
        nscale_sb = const.tile([128, 2], fp32)
        nc.sync.dma_start(nscale_sb[:], nscale[:])
        nbias_sb = const.tile([128, 2], fp32)
        nc.sync.dma_start(nbias_sb[:], nbias[:])
        i128 = const.tile([128, 128], fp16)
        make_identity(nc, i128[:])
        i27 = const.tile([27, 27], fp16)
        make_identity(nc, i27[:])
        i128f = const.tile([128, 128], fp32)
        make_identity(nc, i128f[:])
        sel_sb = const.tile([16, 128], fp32)
        nc.sync.dma_start(sel_sb[:], sel[:])

        # ---- stage A: offset conv -> om_sb [27, P] fp16 ----
        om_sb = const.tile([27, P], fp16)
        psB = ctx.enter_context(tc.tile_pool(name="psB", bufs=1, space="PSUM"))
        psA = ctx.enter_context(tc.tile_pool(name="psA", bufs=1, space="PSUM"))
        if True:
            for s in range(NSLAB):
                pom = psA.tile([27, SLAB * W], fp32, tag="pom")
                first = True
                for kp in range(9):
                    dy, dx = kp // 3, kp % 3
                    for ch in range(2):
                        nc.tensor.matmul(
                            pom[:],
                            lhsT=woff_sb[:, (kp * 2 + ch) * 27:(kp * 2 + ch) * 27 + 27],
                            rhs=xcm_sb[ch][:, s * SLAB + dy: s * SLAB + dy + SLAB,
                                           dx: dx + W],
                            start=first,
                            stop=(kp == 8 and ch == 1),
                        )
                        first = False
                nc.scalar.copy(om_sb[:, s * SLAB * W:(s + 1) * SLAB * W], pom[:])

            # ---- stage B: om -> position-major omT [128, NT*27] fp32 ----
            omT = const.tile([128, NT * 27], fp32)
            pT = psB.tile([128, NT, 28], fp16, tag="pT")
            for t in range(NT):
                nc.tensor.transpose(
                    pT[:, t, 0:27],
                    om_sb[:, t * 128:(t + 1) * 128],
                    i27[:],
                )
            nc.scalar.copy(
                omT[:].rearrange("p (t c) -> p t c", c=27), pT[:, :, 0:27])

        # ---- stage C: coords / weights / indices ----
        def ctile(tag, dtype=fp32, cols=9 * NT):
            return coords.tile([128, cols], dtype, tag=tag, name=tag)

        T = ctile("T", cols=27 * NT)
        nc.vector.tensor_add(T[:], omT[:], base_sb[:])
        T3 = T[:].rearrange("p (t c) -> p t c", c=27)
        SX = T3[:, :, 0:9]
        SY = T3[:, :, 9:18]
        ML = T3[:, :, 18:27]

        FY = ctile("FY")
        nc.vector.tensor_scalar(FY[:], SY, 0.5, MAGIC, OP.subtract, OP.add)
        nc.vector.tensor_scalar(FY[:], FY[:], MAGIC, None, OP.subtract)
        FX = ctile("FX")
        nc.vector.tensor_scalar(FX[:], SX, 0.5, MAGIC, OP.subtract, OP.add)
        nc.vector.tensor_scalar(FX[:], FX[:], MAGIC, None, OP.subtract)

        WY1 = ctile("WY1")
        nc.vector.tensor_tensor(WY1[:], SY, FY[:], OP.subtract)
        WX1 = ctile("WX1")
        nc.vector.tensor_tensor(WX1[:], SX, FX[:], OP.subtract)
        MSK = ctile("MSK")
        nc.scalar.activation(MSK[:], ML, AF.Sigmoid)

        C0Y = ctile("C0Y")
        nc.vector.tensor_scalar(C0Y[:], FY[:], 0.0, 95.0, OP.max, OP.min)
        VY0 = ctile("VY0")
        nc.vector.tensor_tensor(VY0[:], FY[:], C0Y[:], OP.is_equal)
        C1Y = ctile("C1Y")
        nc.vector.tensor_scalar(C1Y[:], FY[:], -1.0, 94.0, OP.max, OP.min)
        VY1 = ctile("VY1")
        nc.vector.tensor_tensor(VY1[:], FY[:], C1Y[:], OP.is_equal)
        Y1C = C1Y
        nc.vector.tensor_scalar(Y1C[:], C1Y[:], 1.0, None, OP.add)

        XADJ = ctile("XADJ")
        nc.vector.tensor_scalar(XADJ[:], FX[:], -1.0, 95.0, OP.max, OP.min)
        C0X = ctile("C0X")
        nc.vector.tensor_scalar(C0X[:], FX[:], 0.0, 95.0, OP.max, OP.min)
        VX0 = ctile("VX0")
        nc.vector.tensor_tensor(VX0[:], FX[:], C0X[:], OP.is_equal)
        C1X = ctile("C1X")
        nc.vector.tensor_scalar(C1X[:], FX[:], -1.0, 94.0, OP.max, OP.min)
        VX1 = ctile("VX1")
        nc.vector.tensor_tensor(VX1[:], FX[:], C1X[:], OP.is_equal)

        WY0M = ctile("WY0M")
        nc.vector.tensor_scalar(WY0M[:], WY1[:], -1.0, 1.0, OP.mult, OP.add)
        nc.vector.tensor_tensor(WY0M[:], WY0M[:], VY0[:], OP.mult)
        nc.vector.tensor_tensor(WY0M[:], WY0M[:], MSK[:], OP.mult)
        WY1M = ctile("WY1M")
        nc.vector.tensor_tensor(WY1M[:], WY1[:], VY1[:], OP.mult)
        nc.vector.tensor_tensor(WY1M[:], WY1M[:], MSK[:], OP.mult)
        WX0V = ctile("WX0V")
        nc.vector.tensor_scalar(WX0V[:], WX1[:], -1.0, 1.0, OP.mult, OP.add)
        nc.vector.tensor_tensor(WX0V[:], WX0V[:], VX0[:], OP.mult)
        WX1V = ctile("WX1V")
        nc.vector.tensor_tensor(WX1V[:], WX1[:], VX1[:], OP.mult)

        W00 = ctile("W00")
        nc.vector.tensor_tensor(W00[:], WY0M[:], WX0V[:], OP.mult)
        W01 = ctile("W01")
        nc.vector.tensor_tensor(W01[:], WY0M[:], WX1V[:], OP.mult)
        W10 = ctile("W10")
        nc.vector.tensor_tensor(W10[:], WY1M[:], WX0V[:], OP.mult)
        W11 = ctile("W11")
        nc.vector.tensor_tensor(W11[:], WY1M[:], WX1V[:], OP.mult)

        # indices: b = rowc*96 + 1 + xadj, layout bC [128, (t, yrow, k)] int32
        B0F = ctile("B0F")
        nc.vector.tensor_scalar(B0F[:], C0Y[:], 96.0, 1.0, OP.mult, OP.add)
        nc.vector.tensor_tensor(B0F[:], B0F[:], XADJ[:], OP.add)
        B1F = ctile("B1F")
        nc.vector.tensor_scalar(B1F[:], Y1C[:], 96.0, 1.0, OP.mult, OP.add)
        nc.vector.tensor_tensor(B1F[:], B1F[:], XADJ[:], OP.add)
        bCf = coords.tile([128, NT, 18], fp32, tag="bCf")
        nc.vector.tensor_copy(bCf[:, :, 0:9], B0F[:].rearrange("p (t k) -> p t k", k=9))
        nc.vector.tensor_copy(bCf[:, :, 9:18], B1F[:].rearrange("p (t k) -> p t k", k=9))

        # ---- wrapped gather-index construction ----
        # dma_gather wants idxs [128, n/16] int16 with idx j at partition
        # j%16 (replicated across the 8 gpsimd core groups); the gather
        # places row j at partition j%128. Our position-major bC holds
        # idx(j) at partition j%128 already, so build the wrap via:
        # PE-transpose bC -> T_sb (list cols on partitions), flatten to a
        # single row, 16-row shift DMA, strided compact to [16, n/16], and a
        # SEL matmul to broadcast to all 128 partitions.
        bC2 = bCf[:].rearrange("p t k -> p (t k)")
        T_sb = coords.tile([72, 9, 128], i16, tag="T_sb")
        for b in range(9):
            pT2 = psB.tile([72, 128], fp32, tag="pT", name="pT2")
            nc.tensor.matmul(pT2[:], lhsT=bC2[:, b * 72:(b + 1) * 72],
                             rhs=i128f[:], is_transpose=True,
                             start=True, stop=True)
            nc.scalar.copy(T_sb[:, b, :], pT2[:])

        W16 = coords.tile([128, 9, 2, 288], i16, tag="W16")
        for b in range(9):
            flat = coords.tile([1, 9232], i16, tag="flat")
            fh = flat[:].tensor
            nc.sync.dma_start(
                bass.AP(fh, flat[:].offset, [[9232, 1], [128, 72], [1, 128]]),
                T_sb[:, b, :])
            nc.vector.memset(flat[0:1, 9216:9232], 0)
            sh = coords.tile([16, 9216], i16, tag="sh")
            nc.sync.dma_start(
                sh[:], bass.AP(fh, flat[:].offset, [[9232, 1], [1, 16], [1, 9216]]))
            f2 = coords.tile([16, 576], fp32, tag="f2")
            nc.vector.tensor_copy(
                f2[:], bass.AP(sh[:].tensor, sh[:].offset,
                               [[9216, 16], [16, 576]]))
            for hh in range(2):
                pW = psB.tile([128, 288], fp32, tag="pT", name="pW")
                nc.tensor.matmul(pW[:], lhsT=sel_sb[:],
                                 rhs=f2[:, hh * 288:(hh + 1) * 288],
                                 start=True, stop=True)
                nc.scalar.copy(W16[:, b, hh, :], pW[:])

        # ---- stages D-G per position tile ----
        psv = ctx.enter_context(tc.tile_pool(name="psv", bufs=2, space="PSUM"))
        pso = ctx.enter_context(tc.tile_pool(name="pso", bufs=2, space="PSUM"))
        xt_pairs = bass.AP(xt[:].tensor, 0, [[256, HW + 1], [1, 2 * CIN]])
        for t in range(NT):
            gt = gpool.tile([128, 18, 2 * CIN], fp16, tag="gt")
            wslice = W16[:].rearrange("p b h f -> p (b h f)")[
                :, t * 144:(t + 1) * 144]
            nc.gpsimd.dma_gather(
                gt[:], xt_pairs, wslice, num_idxs=18 * 128,
                num_idxs_reg=18 * 128, elem_size=2 * CIN, elem_step=CIN,
                single_packet=False)
            if True:
                pv = [psv.tile([128, 9 * 128], fp16, tag="pv", name=f"pv{h}")
                      for h in range(2)]
                for k in range(9):
                    col = t * 9 + k
                    j0 = k
                    j1 = 9 + k
                    m0 = bl.tile([128, CIN], fp16, tag="m0")
                    nc.vector.tensor_scalar(
                        m0[:], gt[:, j0, 0:CIN], W00[:, col:col + 1], None, OP.mult)
                    p0 = bl.tile([128, CIN], fp16, tag="p0")
                    nc.vector.scalar_tensor_tensor(
                        p0[:], gt[:, j0, CIN:2 * CIN], W01[:, col:col + 1], m0[:],
                        OP.mult, OP.add)
                    q0 = bl.tile([128, CIN], fp16, tag="q0")
                    nc.vector.scalar_tensor_tensor(
                        q0[:], gt[:, j1, 0:CIN], W10[:, col:col + 1], p0[:],
                        OP.mult, OP.add)
                    v0 = bl.tile([128, CIN], fp16, tag="v0")
                    nc.vector.scalar_tensor_tensor(
                        v0[:], gt[:, j1, CIN:2 * CIN], W11[:, col:col + 1], q0[:],
                        OP.mult, OP.add)
                    for h in range(2):
                        nc.tensor.matmul(
                            pv[h][:, k * 128:(k + 1) * 128],
                            lhsT=v0[:, h * 128:(h + 1) * 128],
                            rhs=i128[:], is_transpose=True,
                            start=True, stop=True)
                val = [vpool.tile([128, 9 * 128], fp16, tag=f"val{h}",
                                  name=f"val{h}") for h in range(2)]
                for h in range(2):
                    nc.scalar.copy(val[h][:], pv[h][:])

                pout = pso.tile([128, 256], fp32, tag="pout")
                for oh in range(2):
                    idx = 0
                    for h in range(2):
                        for k in range(9):
                            nc.tensor.matmul(
                                pout[:, oh * 128:(oh + 1) * 128],
                                lhsT=w2_sb[:, ((h * 9 + k) * 2 + oh) * 128:
                                           ((h * 9 + k) * 2 + oh) * 128 + 128],
                                rhs=val[h][:, k * 128:(k + 1) * 128],
                                start=(idx == 0), stop=(idx == 17))
                            idx += 1
                for oh in range(2):
                    ot = opool.tile([128, 128], fp32, tag=f"ot{oh}")
                    nc.scalar.activation(
                        ot[:], pout[:, oh * 128:(oh + 1) * 128], AF.Relu,
                        bias=nbias_sb[:, oh:oh + 1], scale=nscale_sb[:, oh:oh + 1])
                    nc.sync.dma_start(
                        out_d[oh * 128:(oh + 1) * 128, t * 128:(t + 1) * 128], ot[:])

    nc.finalize()
    return nc


def _prep_core_inputs(x_img, r0, shared):
    """Per-core input dict. x_img: [256, 96, 96] fp32; r0: first output row."""
    xt = np.zeros((HW + 2, CIN), np.float16)
    xt[1:HW + 1] = x_img.reshape(CIN, HW).T.astype(np.float16)

    xcm = np.zeros((2, 128, NROWS + 2, W + 2), np.float16)
    lo = max(0, r0 - 1)
    hi = min(H, r0 + NROWS + 1)
    for ch in range(2):
        xcm[ch, :, lo - (r0 - 1):hi - (r0 - 1), 1:W + 1] = \
            x_img[ch * 128:(ch + 1) * 128, lo:hi, :].astype(np.float16)

    return {"xt": xt, "xcm": xcm.reshape(2, 128, (NROWS + 2) * (W + 2)), **shared,
            "base": shared["base_by_half"][0 if r0 == 0 else 1]}


def _prep_shared(w_off, b_off, w, gamma, beta, mean, var):
    woff = np.zeros((128, 9 * 2 * 27), np.float16)
    for kp in range(9):
        dy, dx = kp // 3, kp % 3
        for ch in range(2):
            woff[:, (kp * 2 + ch) * 27:(kp * 2 + ch) * 27 + 27] = \
                w_off[:, ch * 128:(ch + 1) * 128, dy, dx].T.astype(np.float16)

    w9 = w.reshape(COUT, CIN, 3, 3).reshape(COUT, CIN, K)
    w2 = np.zeros((128, 2 * 9 * 2 * 128), np.float16)
    for h in range(2):
        for k in range(K):
            for oh in range(2):
                blk = ((h * 9 + k) * 2 + oh) * 128
                w2[:, blk:blk + 128] = \
                    w9[oh * 128:(oh + 1) * 128, h * 128:(h + 1) * 128, k].T.astype(np.float16)

    inv = (gamma / np.sqrt(var + EPS)).astype(np.float32)
    bias2 = (beta - mean * inv).astype(np.float32)
    nscale = np.stack([inv[0:128], inv[128:256]], axis=1).astype(np.float32)
    nbias = np.stack([bias2[0:128], bias2[128:256]], axis=1).astype(np.float32)

    ky = np.repeat(np.arange(-1, 2, dtype=np.float32), 3)
    kx = np.tile(np.arange(-1, 2, dtype=np.float32), 3)
    base_by_half = []
    for half in range(2):
        r0 = half * NROWS
        b = np.zeros((128, NT, 27), np.float32)
        pidx = np.arange(P)
        gy = (r0 + pidx // W).astype(np.float32).reshape(NT, 128)
        gx = (pidx % W).astype(np.float32).reshape(NT, 128)
        for k in range(K):
            b[:, :, k] = (kx[k] + gx + b_off[k]).T
            b[:, :, 9 + k] = (ky[k] + gy + b_off[9 + k]).T
            b[:, :, 18 + k] = b_off[18 + k]
        base_by_half.append(b.reshape(128, NT * 27))

    sel = np.zeros((16, 128), np.float32)
    for q in range(128):
        sel[q % 16, q] = 1.0

    return {"woff": woff, "w2": w2, "nscale": nscale, "nbias": nbias,
            "sel": sel, "base_by_half": base_by_half}


def kernel(x, w_off, b_off, w, gamma, beta, mean, var):
    x = np.asarray(x, np.float32)
    w_off = np.asarray(w_off, np.float32)
    b_off = np.asarray(b_off, np.float32)
    w = np.asarray(w, np.float32)
    gamma = np.asarray(gamma, np.float32)
    beta = np.asarray(beta, np.float32)
    mean = np.asarray(mean, np.float32)
    var = np.asarray(var, np.float32)

    if "nc" not in _PROGRAM_CACHE:
        _PROGRAM_CACHE["nc"] = _build_program()
    nc = _PROGRAM_CACHE["nc"]

    shared = _prep_shared(w_off, b_off, w, gamma, beta, mean, var)
    in_maps = []
    for core in range(8):
        n, half = core // 2, core % 2
        m = _prep_core_inputs(x[n], half * NROWS, shared)
        m.pop("base_by_half", None)
        in_maps.append(m)

    res = run_bass_kernel_spmd(nc, in_maps, core_ids=list(range(8)))
    _PROGRAM_CACHE["last_results"] = res

    out = np.zeros((N, COUT, H, W), np.float32)
    for core in range(8):
        n, half = core // 2, core % 2
        r0 = half * NROWS
        out[n, :, r0:r0 + NROWS, :] = res.results[core]["out"].reshape(COUT, NROWS, W)
    return out
